# revision 33
# baseline (speedup 1.0000x reference)
"""Trainium2 Bass kernel for the attention-like exp/reduce problem.

Math (per batch element b, data-parallel across 8 cores):
    colsum[t,q] = sum_p exp(dec[p] * enc[t,q])
    rowsum[t,q] = sum_r exp(dec[q] * enc[t,r])
    out[q]      = sum_t enc[t,q] * colsum[t,q] / rowsum[t,q]

Instead of materializing the [T,D,D] tensor (8.4M exps/core), both sums are
computed from K=14 shared Chebyshev-node exponential tiles via Gaussian-
weighted barycentric interpolation in the dec variable:

    exp(y*x) = e^{y^2/2} * G_x(y),  G_x(y) = e^{y*x - y^2/2}  (Gaussian in y)
    G_x(y) ~= sum_k l_k(y) * G_x(y_k)   (Chebyshev nodes y_k on dec's range;
                                         G_x is entire & bump-shaped -> fast
                                         convergence, benign conditioning)

With E_k[t,q] = exp(y_k*enc[t,q]), cw_k = e^{-y_k^2/2}, V_q = e^{dec_q^2/2},
barycentric weights au[k,q] = w_k/(dec_q - y_k), S_q = sum_k au[k,q]:

    colsum[t,q] ~= sum_k g_k E_k[t,q],   g_k = cw_k * sum_q l_k(dec_q) V_q
    rowsum[t,q] ~= (V_q/S_q) * sum_k (cw_k H[k,t]) au[k,q],  H[k,t] = sum_r E_k[t,r]

Engine split per node k: ACT computes E_k (f32).  For the first NDV nodes a
single DVE tensor_scalar forms GE_k = g_k*E_k (f32r) with accum_out giving
Hg[t,k] = g_k*H[k,t] for free; for the rest, H comes from the ACT accum_out
and DVE only does the plain g-scale (the cw/g vs cw rescale is folded into a
masked cwg column).  PE accumulates colsum = sum_k GE_k in PSUM via an
identity stationary (f32r, single-pass).  Rowsum is one [K,128]x[K,256] f32
matmul of the rescaled transposed Hg against au.  S/V and the enc multiply
fold into a precomputed encSV tile; the final t-contraction is a ones-column
matmul.  GPSIMD/Pool does no compute (its tensor ops run ~4us each on HW).
"""

import sys

sys.path.insert(0, "/opt/trn_rl_repo")

import numpy as np

import concourse.bacc as bacc
import concourse.tile as tile
from concourse import mybir
from concourse.bass_utils import run_bass_kernel_spmd

# Enable walrus ldw-opt: dedupes back-to-back identical LDWEIGHTS (the colsum
# accumulation reuses one identity stationary across K matmuls).
import concourse.bass_utils as _bu

if not getattr(_bu, "_ldw_opt_patched", False):
    _orig_run_command = _bu.run_command

    def _run_command_ldw(cmd, *a, **kw):
        if isinstance(cmd, list):
            cmd = [
                "--enable-ldw-opt=true" if c == "--enable-ldw-opt=false" else c
                for c in cmd
            ]
        return _orig_run_command(cmd, *a, **kw)

    _bu.run_command = _run_command_ldw
    _bu._ldw_opt_patched = True

try:
    import antenv.axon_hooks  # noqa: F401
except ImportError:
    import types

    import antenv

    _hooks = types.ModuleType("antenv.axon_hooks")
    _hooks.get_axon_ntff_profile_hook = lambda: None
    _hooks.set_axon_ntff_profile_hook = lambda h: None
    sys.modules["antenv.axon_hooks"] = _hooks
    antenv.axon_hooks = _hooks

B, T, D = 8, 128, 256
NCORES = 8
K = 14               # Chebyshev nodes (1st kind)
NDV = 9              # nodes whose H rides the DVE accum-ts; rest use ACT accum
YA, YB = -3.5, 3.5   # node interval (covers the dec value range)
F32 = mybir.dt.float32
F32R = mybir.dt.float32r
EXP = mybir.ActivationFunctionType.Exp
ADD = mybir.AluOpType.add
MULT = mybir.AluOpType.mult
X_AX = mybir.AxisListType.X


def _nodes():
    j = np.arange(K)
    y = 0.5 * (YA + YB) + 0.5 * (YB - YA) * np.cos((2 * j + 1) * np.pi / (2 * K))
    w = (-1.0) ** j * np.sin((2 * j + 1) * np.pi / (2 * K))
    cw = np.exp(-0.5 * y * y)
    return y.astype(np.float32), w.astype(np.float32), cw.astype(np.float32)


Y_NODES, W_BARY, CW_NODES = _nodes()

# blob_f [128, CF]: f32 constants
#   [0:128)  ident          [128] onescol        [129:129+K) nyb (-y bcast)
#   [129+K:129+2K) wb       [129+2K] nycv        [130+2K] wcv
#   [131+2K] cwcv           [132+2K] mask(k<NDV) [133+2K] imask
#   [134+2K:134+3K) onesrowK (row 0 only = 1)
CF_ONES = 128
CF_NYB = 129
CF_WB = 129 + K
CF_NYCV = 129 + 2 * K
CF_WCV = 130 + 2 * K
CF_CWCV = 131 + 2 * K
CF_MASK = 132 + 2 * K
CF_IMASK = 133 + 2 * K
CF_ONESROWK = 134 + 2 * K
CF_ONESROW = 134 + 3 * K
CF_ONES128 = 262 + 3 * K
CF_COLS = 390 + 3 * K

# blob_c [128, CC]: small f32 constants needed early (g chain)
#   [0:K) nyb   [K:2K) wb   [2K:3K) cwrow (row 0)   [3K] nycv  [3K+1] wcv
#   [3K+2] cwcv  [3K+3] mask  [3K+4] imask  [3K+5] onescol
CC_NYB = 0
CC_WB = K          # [K:3K) = w duplicated for both halves
CC_CWROW = 3 * K
CC_NYCV = 4 * K
CC_WCV = 4 * K + 1
CC_CWCV = 4 * K + 2
CC_MASK = 4 * K + 3
CC_IMASK = 4 * K + 4
CC_ONES = 4 * K + 5
CC_CWB2 = 4 * K + 6          # [128, 2K] cw duplicated both halves
CC_NYROW = 6 * K + 6         # row 0 = -y_k
CC_ONES256 = 7 * K + 6       # row 0 = 1.0 x 256
CC_COLS = 7 * K + 262

# blob_r [128, CR]: f32r constants
#   [0:128) identr   [128] onescolr   [129:257) onesrow (row 0 = 1)
#   [257:257+K) cwrow (row 0 = cw)
CR_ONESCOL = 128
CR_ONESROW = 129
CR_CWROW = 257
CR_COLS = 257 + K


def _blob_f():
    bf = np.zeros((128, CF_COLS), dtype=np.float32)
    bf[:, :128] = np.eye(128, dtype=np.float32)
    bf[:, CF_ONES] = 1.0
    bf[:, CF_NYB:CF_NYB + K] = -Y_NODES[None, :]
    bf[:, CF_WB:CF_WB + K] = W_BARY[None, :]
    bf[:K, CF_NYCV] = -Y_NODES
    bf[:K, CF_WCV] = W_BARY
    bf[:K, CF_CWCV] = CW_NODES
    bf[:NDV, CF_MASK] = 1.0
    bf[NDV:K, CF_IMASK] = 1.0
    bf[0, CF_ONESROWK:CF_ONESROWK + K] = 1.0
    bf[0, CF_ONESROW:CF_ONESROW + 128] = 1.0
    bf[:, CF_ONES128:CF_ONES128 + 128] = 1.0
    return bf


def _blob_c():
    bc = np.zeros((128, CC_COLS), dtype=np.float32)
    bc[:, CC_NYB:CC_NYB + K] = -Y_NODES[None, :]
    bc[:, CC_WB:CC_WB + K] = W_BARY[None, :]
    bc[:, CC_WB + K:CC_WB + 2 * K] = W_BARY[None, :]
    bc[0, CC_CWROW:CC_CWROW + K] = CW_NODES
    bc[:K, CC_NYCV] = -Y_NODES
    bc[:K, CC_WCV] = W_BARY
    bc[:K, CC_CWCV] = CW_NODES
    bc[:NDV, CC_MASK] = 1.0
    bc[NDV:K, CC_IMASK] = 1.0
    bc[:, CC_ONES] = 1.0
    bc[:, CC_CWB2:CC_CWB2 + K] = CW_NODES[None, :]
    bc[:, CC_CWB2 + K:CC_CWB2 + 2 * K] = CW_NODES[None, :]
    bc[0, CC_NYROW:CC_NYROW + K] = -Y_NODES
    bc[0, CC_ONES256:CC_ONES256 + 256] = 1.0
    return bc


def _blob_r():
    br = np.zeros((128, CR_COLS), dtype=np.float32)
    br[:, :128] = np.eye(128, dtype=np.float32)
    br[:, CR_ONESCOL] = 1.0
    br[0, CR_ONESROW:CR_ONESROW + 128] = 1.0
    br[0, CR_CWROW:CR_CWROW + K] = CW_NODES
    return br


def build_nc():
    nc = bacc.Bacc("TRN2")
    enc = nc.dram_tensor("enc", [T, D], F32, kind="ExternalInput").ap()
    decq = nc.dram_tensor("decq", [128, 2], F32, kind="ExternalInput").ap()
    decrow = nc.dram_tensor("decrow", [1, D], F32, kind="ExternalInput").ap()
    blob_f = nc.dram_tensor("blob_f", [128, CF_COLS], F32, kind="ExternalInput").ap()
    blob_c = nc.dram_tensor("blob_c", [128, CC_COLS], F32, kind="ExternalInput").ap()
    blob_r = nc.dram_tensor("blob_r", [128, CR_COLS], F32R, kind="ExternalInput").ap()
    out = nc.dram_tensor("out", [1, D], F32, kind="ExternalOutput").ap()

    with tile.TileContext(nc) as tc:
        with (
            tc.tile_pool(name="const", bufs=1) as cp,
            tc.tile_pool(name="e", bufs=10) as ep,
            tc.tile_pool(name="ge", bufs=4) as gep,
            tc.tile_pool(name="cacc", bufs=1, space="PSUM") as caccp,
            tc.tile_pool(name="rsb", bufs=1, space="PSUM") as rsp,
            tc.tile_pool(name="svbc", bufs=1, space="PSUM") as svbcp,
            tc.tile_pool(name="psmall", bufs=1, space="PSUM") as psp,
        ):
            # ---- ACT table warmup (no input deps; hides the Exp table load).
            # zt stays all-zero and doubles as an explicit bias AP for every
            # activation, avoiding a late const-tensor dependency. ----
            zt = cp.tile([128, 1], F32, tag="zt")
            nc.vector.memset(zt[:], 0.0)
            warm = cp.tile([128, 1], F32, tag="warm")
            nc.scalar.activation(warm[:], zt[:], EXP, bias=zt[:, 0:1])

            # ---- input DMAs: enc first on SP (gates the E loop); dec and
            # the small early-needed consts on the ACT queue ----
            enc_sb = cp.tile([T, D], F32, tag="enc")
            nc.sync.dma_start(enc_sb[:], enc)
            drow = cp.tile([1, D], F32, tag="drow")
            nc.sync.dma_start(drow[:], decrow)
            dec_t2 = cp.tile([128, 2], F32, tag="decq")
            nc.scalar.dma_start(dec_t2[:], decq)
            bc = cp.tile([128, CC_COLS], F32, tag="bc")
            nc.scalar.dma_start(bc[:], blob_c)
            br = cp.tile([128, CR_COLS], F32R, tag="br")
            nc.sync.dma_start(br[:], blob_r)
            bf = cp.tile([128, CF_COLS], F32, tag="bf")
            nc.gpsimd.dma_start(bf[:], blob_f)

            dec_sb = dec_t2[:]
            ident = bf[:, 0:128]
            onescol = bf[:, CF_ONES:CF_ONES + 1]
            nyb = bc[:, CC_NYB:CC_NYB + K]
            wb2 = bc[:, CC_WB:CC_WB + 2 * K]
            cwrow_c = bc[0:1, CC_CWROW:CC_CWROW + K]
            nycv = bc[0:K, CC_NYCV:CC_NYCV + 1]
            wcv = bc[0:K, CC_WCV:CC_WCV + 1]
            cwcv = bc[0:K, CC_CWCV:CC_CWCV + 1]
            maskcv = bc[0:K, CC_MASK:CC_MASK + 1]
            imaskcv = bc[0:K, CC_IMASK:CC_IMASK + 1]
            onescol_c = bc[:, CC_ONES:CC_ONES + 1]
            cwb2 = bc[:, CC_CWB2:CC_CWB2 + 2 * K]
            nyrowK = bc[0:1, CC_NYROW:CC_NYROW + K]
            ones256 = bc[0:1, CC_ONES256:CC_ONES256 + 256]
            onesrowK = bf[0:1, CF_ONESROWK:CF_ONESROWK + K]
            onesrow_f = bf[0:1, CF_ONESROW:CF_ONESROW + 128]
            ones128 = bf[:, CF_ONES128:CF_ONES128 + 128]
            identr = br[:, 0:128]
            onescol_r = br[:, CR_ONESCOL:CR_ONESCOL + 1]
            onesrow_r = br[0:1, CR_ONESROW:CR_ONESROW + 128]
            cwrow_r = br[0:1, CR_CWROW:CR_CWROW + K]

            # ---- q-partition g chain (tiny ops; feeds gbc for the main loop).
            # The barycentric chain comes first in DVE order; d22/vv (needed
            # only from the vsc2 stage) are emitted after it. ----
            d22 = cp.tile([128, 2], F32, tag="d22")
            nd22 = cp.tile([128, 4], F32, tag="nd22")
            vv = cp.tile([128, 4], F32, tag="vv")  # [e^{+d^2/2} | e^{-d^2/2}]
            dd2 = cp.tile([128, 2 * K], F32, tag="dd2")
            rec2 = cp.tile([128, 2 * K], F32, tag="rec2")
            au2 = cp.tile([128, 2 * K], F32, tag="au2")
            s2 = cp.tile([128, 2], F32, tag="s2")
            srec2 = cp.tile([128, 2], F32, tag="srec2")
            vsc2 = cp.tile([128, 2], F32, tag="vsc2")
            sv2 = cp.tile([128, 2], F32, tag="sv2")
            t5 = cp.tile([128, 2 * K], F32, tag="t5")
            for h in range(2):
                sl = slice(h * K, (h + 1) * K)
                nc.vector.tensor_scalar(
                    dd2[:, sl], nyb, dec_sb[:, h:h + 1], None, op0=ADD
                )
            nc.vector.reciprocal_approx_fast(rec2[:], dd2[:])
            nc.vector.tensor_tensor(au2[:], rec2[:], wb2, op=MULT)
            nc.vector.tensor_reduce(
                s2[:], au2[:].rearrange("p (h k) -> p h k", k=K),
                axis=X_AX, op=ADD,
            )
            nc.vector.tensor_tensor(d22[:], dec_sb, dec_sb, op=MULT)
            v2 = vv[:, 0:2]
            vinv2 = vv[:, 2:4]
            nc.scalar.activation(v2, d22[:], EXP, scale=0.5, bias=zt[:, 0:1])
            nc.scalar.activation(vinv2, d22[:], EXP, scale=-0.5, bias=zt[:, 0:1])
            nc.vector.reciprocal_approx_fast(srec2[:], s2[:])
            nc.vector.tensor_tensor(vsc2[:], v2, srec2[:], op=MULT)   # V/S
            for h in range(2):
                sl = slice(h * K, (h + 1) * K)
                nc.vector.tensor_scalar(
                    t5[:, sl], au2[:, sl], vsc2[:, h:h + 1], None, op0=MULT
                )
            # gbc[t, k] = g_k = cw_k sum_q l_k(dec_q) V_q, broadcast to all
            # partitions in one PE stage (ones128 stationary, both halves
            # accumulated).
            psA = psp.tile([128, 512], F32, tag="psA")
            t5c = cp.tile([128, 2 * K], F32, tag="t5c")
            nc.vector.tensor_tensor(t5c[:], t5[:], cwb2, op=MULT)
            gbc_ps = psA[:, 16:16 + K]
            nc.tensor.matmul(gbc_ps, ones128, t5c[:, 0:K], start=True, stop=False)
            nc.tensor.matmul(
                gbc_ps, ones128, t5c[:, K:2 * K], start=False, stop=True
            )
            gbc = cp.tile([128, K], F32, tag="gbcsb")
            nc.vector.tensor_copy(gbc[:], gbc_ps)

            # tail-prep tiles (chains emitted interleaved inside the loop)
            svrow = cp.tile([1, D], F32R, tag="svrow")
            svbc_ps = svbcp.tile([128, D], F32, tag="svbc")
            encsv = cp.tile([T, D], F32, tag="encsv")
            decbc_ps = psp.tile([K, 256], F32, tag="psB")
            ddk = cp.tile([K, D], F32, tag="ddk")
            reck = cp.tile([K, D], F32, tag="reck")
            auk = cp.tile([K, D], F32R, tag="auk")
            gcol = cp.tile([K, 1], F32, tag="gcol")
            grec = cp.tile([K, 1], F32, tag="grec")
            gsel = cp.tile([K, 1], F32, tag="gsel")
            gsel2 = cp.tile([K, 1], F32, tag="gsel2")
            cwg = cp.tile([K, 1], F32, tag="cwg")

            def emit_auk():
                # ddk[k,q] = dec_q - y_k accumulated directly on PE:
                # ones_k (x) dec_row  +  (-y)_k (x) ones_256
                nc.tensor.matmul(
                    decbc_ps[:], onesrowK, drow[:], start=True, stop=False
                )
                nc.tensor.matmul(
                    decbc_ps[:], nyrowK, ones256, start=False, stop=True
                )
                nc.vector.reciprocal_approx_fast(reck[:], decbc_ps[:])
                nc.vector.tensor_scalar(auk[:], reck[:], wcv, None, op0=MULT)

            def emit_cwg():
                # cwg: cw/g for DVE-accum nodes (Hg holds g*H), cw for
                # ACT-accum nodes (plain H); selected via constant masks.
                gT_ps = psA[0:K, 288:416]
                nc.tensor.transpose(gT_ps, gbc[:], ident)
                nc.vector.reciprocal_approx_fast(grec[:], gT_ps[:, 0:1])
                nc.vector.scalar_tensor_tensor(
                    gsel2[:], grec[:], maskcv, imaskcv, op0=MULT, op1=ADD
                )
                nc.vector.tensor_scalar(cwg[:], gsel2[:], cwcv, None, op0=MULT)

            def emit_sv():
                # SV broadcast + encSV
                nc.vector.tensor_tensor(sv2[:], s2[:], vinv2, op=MULT)  # S/V
                for h in range(2):
                    svT_ps = psA[0:1, 32 + h * 128:32 + (h + 1) * 128]
                    nc.tensor.transpose(svT_ps, sv2[:, h:h + 1], ident)
                nc.vector.tensor_copy(svrow[:], psA[0:1, 32:288])
                nc.tensor.matmul(
                    svbc_ps[:], onesrow_r, svrow[:], start=True, stop=True
                )
                nc.vector.tensor_tensor(
                    encsv[:], enc_sb[:], svbc_ps[:], op=MULT
                )

            # ---- main loop: E_k -> GE_k (+H accum) -> PSUM colsum accumulate ----
            hg = cp.tile([128, K], F32, tag="hg")
            cacc_ps = caccp.tile([T, D], F32, tag="cacc")
            lp = nc.allow_low_precision(
                reason="f32r accum target is bit-identical f32"
            )
            lp.__enter__()
            for k in range(K):
                e_t = ep.tile([T, D], F32, tag="e")
                if k < NDV:
                    nc.scalar.activation(
                        e_t[:], enc_sb[:], EXP, scale=float(Y_NODES[k]),
                        bias=zt[:, 0:1],
                    )
                else:
                    nc.scalar.activation(
                        e_t[:], enc_sb[:], EXP, scale=float(Y_NODES[k]),
                        bias=zt[:, 0:1], accum_out=hg[:, k:k + 1],
                    )
                ge_t = gep.tile([T, D], F32R, tag="ge")
                if k < NDV:
                    nc.vector.tensor_scalar(
                        ge_t[:], e_t[:], gbc[:, k:k + 1], 0.0, op0=MULT, op1=ADD,
                        accum_out=hg[:, k:k + 1],
                    )
                else:
                    nc.vector.tensor_scalar(
                        ge_t[:], e_t[:], gbc[:, k:k + 1], None, op0=MULT
                    )
                nc.tensor.matmul(
                    cacc_ps[:], identr, ge_t[:],
                    start=(k == 0), stop=(k == K - 1),
                )
                if k == 0:
                    emit_auk()
                elif k == 2:
                    emit_cwg()
                elif k == 4:
                    emit_sv()

            lp.__exit__(None, None, None)

            # ---- rowsum interp + combine ----
            hgT_ps = psA[0:K, 288:416]
            nc.tensor.transpose(hgT_ps, hg[:], ident)
            htw = cp.tile([K, 128], F32R, tag="htw")
            nc.vector.tensor_scalar(htw[:], hgT_ps, cwg[:], None, op0=MULT)
            rsA = rsp.tile([128, 512], F32, tag="rsA")
            rs_ps = rsA[:, 0:256]
            rrec = cp.tile([T, D], F32, tag="rrec")
            c1 = cp.tile([T, D], F32, tag="c1")
            contrib = cp.tile([T, D], F32R, tag="contrib")
            for h in range(2):
                hs = slice(h * 128, (h + 1) * 128)
                nc.tensor.matmul(
                    rs_ps[:, hs], htw[:], auk[:, hs],
                    start=True, stop=True,
                )
                nc.vector.reciprocal_approx_fast(rrec[:, hs], rs_ps[:, hs])
                nc.vector.scalar_tensor_tensor(
                    c1[:, hs], cacc_ps[:, hs], 1.0, rrec[:, hs],
                    op0=MULT, op1=MULT,
                )
                nc.vector.tensor_tensor(
                    contrib[:, hs], c1[:, hs], encsv[:, hs], op=MULT
                )
            fin_ps = rsA[0:1, 256:512]
            nc.tensor.matmul(fin_ps, onescol_r, contrib[:], start=True, stop=True)
            out_sb = cp.tile([1, D], F32, tag="outsb")
            nc.vector.tensor_copy(out_sb[:], fin_ps)
            nc.sync.dma_start(out, out_sb[:])
    nc.compile()
    return nc


_NC_CACHE = None


def _get_nc():
    global _NC_CACHE
    if _NC_CACHE is None:
        _NC_CACHE = build_nc()
    return _NC_CACHE


def make_in_maps(dec_t: np.ndarray, enc_out: np.ndarray):
    bf = _blob_f()
    bc = _blob_c()
    br = _blob_r()
    in_maps = []
    for b in range(B):
        dec2 = np.stack([dec_t[b, :128], dec_t[b, 128:]], axis=1)
        in_maps.append(
            {
                "enc": np.ascontiguousarray(enc_out[b]).astype(np.float32),
                "decq": np.ascontiguousarray(dec2).astype(np.float32),
                "decrow": np.ascontiguousarray(
                    dec_t[b][None, :]
                ).astype(np.float32),
                "blob_f": bf,
                "blob_c": bc,
                "blob_r": br,
            }
        )
    return in_maps


def run(dec_t: np.ndarray, enc_out: np.ndarray, **kwargs):
    """Run on all 8 cores; returns ([B, D] output, BassKernelResults)."""
    nc = _get_nc()
    res = run_bass_kernel_spmd(
        nc, make_in_maps(dec_t, enc_out), core_ids=list(range(NCORES)), **kwargs
    )
    out = np.stack([np.asarray(r["out"]).reshape(D) for r in res.results], axis=0)
    return out.astype(np.float32), res


def kernel(dec_t: np.ndarray, enc_out: np.ndarray) -> np.ndarray:
    dec_t = np.asarray(dec_t, dtype=np.float32)
    enc_out = np.asarray(enc_out, dtype=np.float32)
    out, _ = run(dec_t, enc_out)
    return out
